# revision 5
# baseline (speedup 1.0000x reference)
"""CausalGNN forward on 8 Trainium2 NeuronCores (Bass/Tile).

Math (PyG-style GCN, 3 layers, BN training-mode, residuals):
    deg[v] = 1 + #{edges with dst=v};  dis = deg^-1/2
    per layer i:  h = x @ W_i;  agg[v] = sum_{e=(u,v)} dis_u dis_v h[u]
                  + dis_v^2 h[v]   (+ bias b_i, which BN cancels exactly)
                  y = BN(agg) (batch stats over all nodes), ReLU if i<2
                  x = y (i=0) or x + y (i>0)

Sharding: nodes (and the dst side of aggregation) are partitioned across 8
cores in contiguous ranges; edges live with their dst core, bucketed into
128-node chunks; self-edges are appended so the self term rides the same
path. The dis_u factor is folded into the gather table (h' = dis*h, exact:
row scaling commutes with x @ W), the dis_v factor is a per-column scale
applied once per chunk after PSUM accumulation.

Per layer, per core: h' for own nodes -> AllGather table -> per edge-tile:
indirect-gather h'[src] (128 rows), build a one-hot [edge, dst] on VectorE,
accumulate aggT[feature, dst] on TensorE in PSUM -> column scale + BN stats
-> AllReduce stats -> scale/shift (+ReLU) on ScalarE -> residual.

Everything on device except index bookkeeping: the host only buckets/sorts/
pads edge lists, transposes input/output layouts, and slices per-core
shards. Degrees, dis, norms, matmuls, BN are all computed on device.
"""
import sys
sys.path.insert(0, "/opt/trn_rl_repo")

import numpy as np

import concourse.bass as bass
import concourse.tile as tile
from concourse import bacc, mybir

f32 = mybir.dt.float32
i32 = mybir.dt.int32

P = 128
CORES = 8
L = 3
EPS = 1e-5


# ---------------------------------------------------------------- host prep

def _prep(x, edge_index):
    """Bucket edges by (core, chunk), append self-edges, pad to 128-tiles.

    Returns per-core arrays + the chunk tile counts (shared across cores).
    """
    N, D = x.shape
    E = edge_index.shape[1]
    n_own = (N + CORES - 1) // CORES            # nodes per core (last short)
    n_pad = ((n_own + P - 1) // P) * P          # padded to chunk multiple
    n_chunks = n_pad // P

    src = edge_index[0].astype(np.int64)
    dst = edge_index[1].astype(np.int64)

    # global padded-table row of node n (tables are [CORES*n_pad, D])
    def table_row(n):
        c = n // n_own
        return c * n_pad + (n - c * n_own)

    core_of = dst // n_own
    local = dst - core_of * n_own
    chunk_of = local // P
    dst_rel = local % P

    # self-edges: every real node, plus pad slots (src -> own row 0) so that
    # deg >= 1 everywhere and no inf/NaN enters the pipeline
    counts = np.zeros((CORES, n_chunks), np.int64)
    np.add.at(counts, (core_of, chunk_of), 1)
    counts += P  # one self-edge per slot in every chunk (incl. pad slots)

    tiles_per_chunk = ((counts.max(axis=0) + P - 1) // P).astype(np.int64)
    tile_base = np.concatenate([[0], np.cumsum(tiles_per_chunk)])
    NT = int(tile_base[-1])

    src_arr = np.zeros((CORES, P, NT), np.int32)      # table rows to gather
    rel_arr = np.full((CORES, P, NT), -1.0, np.float32)  # dst col or -1

    fill = np.zeros((CORES, n_chunks), np.int64)

    def put(c, ch, s_row, r):
        j = fill[c, ch]
        fill[c, ch] = j + 1
        t = tile_base[ch] + j // P
        p = j % P
        src_arr[c, p, t] = s_row
        rel_arr[c, p, t] = r

    # self-edges first (also covers pad slots)
    for c in range(CORES):
        base = c * n_own
        for ch in range(n_chunks):
            for r in range(P):
                n_local = ch * P + r
                if base + n_local < N and n_local < n_own:
                    put(c, ch, c * n_pad + n_local, r)
                else:
                    put(c, ch, c * n_pad, r)  # pad slot: gather own row 0
    # real edges (vectorized fill)
    order = np.lexsort((chunk_of, core_of))
    so, co, cho, dro = (src[order], core_of[order], chunk_of[order],
                        dst_rel[order])
    rows = table_row(so)
    grp = co * n_chunks + cho
    # positions within each (core, chunk) group, offset by current fill
    starts = np.searchsorted(grp, np.arange(CORES * n_chunks))
    pos = np.arange(E) - starts[grp] + fill.ravel()[grp]
    t_idx = tile_base[cho] + pos // P
    p_idx = pos % P
    src_arr[co, p_idx, t_idx] = rows
    rel_arr[co, p_idx, t_idx] = dro

    # per-core transposed, padded inputs
    xT = np.zeros((CORES, D, n_pad), np.float32)
    for c in range(CORES):
        lo, hi = c * n_own, min((c + 1) * n_own, N)
        xT[c, :, :hi - lo] = x[lo:hi].T
    return (xT, src_arr, rel_arr, tiles_per_chunk.astype(int), n_own, n_pad,
            n_chunks, NT)


# ------------------------------------------------------------- device build

def _build(D, n_pad, n_chunks, NT, tiles_per_chunk, n_real_last, N_total):
    """Build the SPMD Bass program (same for all cores)."""
    import os
    STAGE = int(os.environ.get("KERNEL_STAGE", "4"))
    nc = bacc.Bacc("TRN2", target_bir_lowering=False, debug=False,
                   num_devices=CORES)
    TBL = CORES * n_pad

    xT_in = nc.dram_tensor("xT_in", [D, n_pad], f32, kind="ExternalInput")
    src_in = nc.dram_tensor("src_in", [P, NT], i32, kind="ExternalInput")
    rel_in = nc.dram_tensor("rel_in", [P, NT], f32, kind="ExternalInput")
    Ws_in = nc.dram_tensor("Ws_in", [L * D, D], f32, kind="ExternalInput")
    gb_in = nc.dram_tensor("gb_in", [D, 2 * L], f32, kind="ExternalInput")
    out_ext = nc.dram_tensor("out", [D, n_pad], f32, kind="ExternalOutput")

    h_own = nc.dram_tensor("h_own", [n_pad, D], f32)
    h_tbl = nc.dram_tensor("h_tbl", [TBL, D], f32)
    h_gat = nc.dram_tensor("h_gat", [TBL, D], f32, kind="ExternalOutput")
    st_in = nc.dram_tensor("st_in", [P, 2], f32)
    st_out = nc.dram_tensor("st_out", [P, 2], f32)

    RG = [list(range(CORES))]
    AOP = mybir.AluOpType

    with tile.TileContext(nc) as tc:
        with tc.tile_pool(name="big", bufs=1) as big, \
             tc.tile_pool(name="sm", bufs=1) as sm, \
             tc.tile_pool(name="gat", bufs=12) as gat, \
             tc.tile_pool(name="oh", bufs=6) as ohp, \
             tc.tile_pool(name="work", bufs=3) as wk, \
             tc.tile_pool(name="ps", bufs=2, space="PSUM") as ps, \
             tc.tile_pool(name="psd", bufs=2, space="PSUM") as psd:

            # ---------------- persistent SBUF state
            xT = big.tile([D, n_pad], f32)
            nc.sync.dma_start(out=xT[:], in_=xT_in[:, :])
            src_sb = big.tile([P, NT], i32)
            nc.sync.dma_start(out=src_sb[:], in_=src_in[:, :])
            rel_sb = big.tile([P, NT], f32)
            nc.sync.dma_start(out=rel_sb[:], in_=rel_in[:, :])
            Ws_sb = sm.tile([D, L * D], f32)
            for i in range(L):
                nc.sync.dma_start(out=Ws_sb[:, i * D:(i + 1) * D],
                                  in_=Ws_in[i * D:(i + 1) * D, :])
            gb_sb = sm.tile([D, 2 * L], f32)
            nc.sync.dma_start(out=gb_sb[:], in_=gb_in[:, :])

            iota_i = sm.tile([P, P], i32)
            nc.gpsimd.iota(iota_i[:], pattern=[[1, P]], base=0,
                           channel_multiplier=0)
            iota_f = sm.tile([P, P], f32)
            nc.vector.tensor_copy(iota_f[:], iota_i[:])
            iota_col_i = sm.tile([P, P], i32)
            nc.gpsimd.iota(iota_col_i[:], pattern=[[1, P]], base=0,
                           channel_multiplier=1)
            iota_col = sm.tile([P, 1], f32)
            nc.vector.tensor_copy(iota_col[:], iota_col_i[:, 0:1])
            ones_col = sm.tile([P, 1], f32)
            nc.vector.memset(ones_col[:], 1.0)
            ones_sq = sm.tile([P, P], f32)
            nc.vector.memset(ones_sq[:], 1.0)

            dis_col = sm.tile([P, n_chunks], f32)   # dis, node-major cols
            dis_bc = big.tile([P, n_pad], f32)      # dis bcast over rows
            agg = big.tile([D, n_pad], f32)         # aggT per layer
            n_own_cols = (n_chunks - 1) * P + n_real_last
            if n_own_cols < n_pad:
                nc.vector.memset(agg[:, n_own_cols:], 0.0)
            slots = sm.tile([P, 2 * n_chunks], f32)  # per-chunk sums/sumsq
            stat = sm.tile([P, 8], f32)              # small scratch columns

            tb = np.concatenate([[0], np.cumsum(tiles_per_chunk)]).astype(int)

            # ---------------- one-time: degrees -> dis -> dis broadcast
            for ch in range(n_chunks):
                dps = psd.tile([P, 1], f32, space="PSUM", tag="deg")
                for t in range(tb[ch], tb[ch + 1]):
                    oht = ohp.tile([P, P], f32, tag="oh")
                    nc.vector.tensor_scalar(
                        out=oht[:], in0=iota_f[:],
                        scalar1=rel_sb[:, t:t + 1], scalar2=None,
                        op0=AOP.is_equal)
                    nc.tensor.matmul(out=dps[:], lhsT=oht[:], rhs=ones_col[:],
                                     start=(t == tb[ch]),
                                     stop=(t == tb[ch + 1] - 1))
                # dis = 1/sqrt(deg)
                nc.vector.reciprocal(stat[:, 0:1], dps[:])
                nc.scalar.sqrt(dis_col[:, ch:ch + 1], stat[:, 0:1])
                # dis broadcast to all partitions: ones128 @ diag(dis)
                diag = wk.tile([P, P], f32, tag="diag")
                nc.vector.tensor_scalar(
                    out=diag[:], in0=iota_f[:], scalar1=iota_col[:],
                    scalar2=dis_col[:, ch:ch + 1],
                    op0=AOP.is_equal, op1=AOP.mult)
                bps = psd.tile([P, P], f32, space="PSUM", tag="bc")
                nc.tensor.matmul(out=bps[:], lhsT=ones_sq[:], rhs=diag[:],
                                 start=True, stop=True)
                nc.scalar.copy(dis_bc[:, ch * P:(ch + 1) * P], bps[:])

            # ---------------- layers
            inv_n = 1.0 / float(N_total)
            if STAGE == 1:
                nc.scalar.copy(xT[:, 0:n_chunks], dis_col[:, 0:n_chunks])
            for i in range(range(0) and 0 or (L if STAGE >= 2 else 0)):
                # h' = dis * (x @ W_i), written row-major into own table rows
                for ch in range(n_chunks):
                    hps = ps.tile([P, D], f32, space="PSUM", tag="h")
                    nc.tensor.matmul(out=hps[:],
                                     lhsT=xT[:, ch * P:(ch + 1) * P],
                                     rhs=Ws_sb[:, i * D:(i + 1) * D],
                                     start=True, stop=True)
                    hsb = wk.tile([P, D], f32, tag="hsb")
                    nc.scalar.mul(out=hsb[:], in_=hps[:],
                                  mul=dis_col[:, ch:ch + 1])
                    nc.sync.dma_start(out=h_own[ch * P:(ch + 1) * P, :],
                                      in_=hsb[:])
                nc.gpsimd.collective_compute(
                    "AllGather", AOP.bypass, replica_groups=RG,
                    ins=[h_own[:, :]], outs=[h_tbl[:, :]])
                nc.sync.dma_start(out=h_gat[:, :], in_=h_tbl[:, :])
                if STAGE == 2:
                    continue

                # edge phase: gather + one-hot matmul, chunk accumulation
                for ch in range(n_chunks):
                    aps = ps.tile([D, P], f32, space="PSUM", tag="agg")
                    for t in range(tb[ch], tb[ch + 1]):
                        g = gat.tile([P, D], f32, tag="g")
                        nc.gpsimd.indirect_dma_start(
                            out=g[:], out_offset=None,
                            in_=h_gat[:, :],
                            in_offset=bass.IndirectOffsetOnAxis(
                                ap=src_sb[:, t:t + 1], axis=0))
                        oht = ohp.tile([P, P], f32, tag="oh")
                        nc.vector.tensor_scalar(
                            out=oht[:], in0=iota_f[:],
                            scalar1=rel_sb[:, t:t + 1], scalar2=None,
                            op0=AOP.is_equal)
                        nc.tensor.matmul(out=aps[:], lhsT=g[:], rhs=oht[:],
                                         start=(t == tb[ch]),
                                         stop=(t == tb[ch + 1] - 1))
                    # column scale by dis_dst; accumulate BN sums
                    w = P if ch < n_chunks - 1 else n_real_last
                    nc.vector.tensor_tensor(
                        out=agg[:, ch * P:ch * P + w],
                        in0=aps[:, 0:w],
                        in1=dis_bc[:, ch * P:ch * P + w],
                        op=AOP.mult)
                    nc.vector.tensor_reduce(
                        out=slots[:, ch:ch + 1],
                        in_=agg[:, ch * P:ch * P + w],
                        axis=mybir.AxisListType.X, op=AOP.add)
                    sq = wk.tile([P, P], f32, tag="sq")
                    nc.vector.tensor_tensor(
                        out=sq[:, 0:w], in0=agg[:, ch * P:ch * P + w],
                        in1=agg[:, ch * P:ch * P + w], op=AOP.mult)
                    nc.vector.tensor_reduce(
                        out=slots[:, n_chunks + ch:n_chunks + ch + 1],
                        in_=sq[:, 0:w],
                        axis=mybir.AxisListType.X, op=AOP.add)

                if STAGE == 3:
                    for ch in range(n_chunks):
                        s = slice(ch * P, (ch + 1) * P)
                        nc.vector.tensor_copy(xT[:, s], agg[:, s])
                    continue
                # stats: reduce chunk slots, AllReduce, scale/shift
                nc.vector.tensor_reduce(
                    out=stat[:, 0:1], in_=slots[:, 0:n_chunks],
                    axis=mybir.AxisListType.X, op=AOP.add)
                nc.vector.tensor_reduce(
                    out=stat[:, 1:2], in_=slots[:, n_chunks:2 * n_chunks],
                    axis=mybir.AxisListType.X, op=AOP.add)
                sin = wk.tile([P, 2], f32, tag="stin")
                nc.vector.tensor_copy(sin[:], stat[:, 0:2])
                nc.sync.dma_start(out=st_in[:, :], in_=sin[:])
                nc.gpsimd.collective_compute(
                    "AllReduce", AOP.add, replica_groups=RG,
                    ins=[st_in[:, :]], outs=[st_out[:, :]])
                sout = wk.tile([P, 2], f32, tag="stout")
                nc.sync.dma_start(out=sout[:], in_=st_out[:, :])
                # mean, var, scale = gamma*rsqrt(var+eps), shift = beta-sc*mean
                nc.vector.tensor_scalar(out=stat[:, 2:3], in0=sout[:, 0:1],
                                        scalar1=inv_n, scalar2=None,
                                        op0=AOP.mult)           # mean
                nc.vector.tensor_scalar(out=stat[:, 3:4], in0=sout[:, 1:2],
                                        scalar1=inv_n, scalar2=None,
                                        op0=AOP.mult)           # E[x^2]
                nc.vector.tensor_tensor(out=stat[:, 4:5], in0=stat[:, 2:3],
                                        in1=stat[:, 2:3], op=AOP.mult)
                nc.vector.tensor_tensor(out=stat[:, 4:5], in0=stat[:, 3:4],
                                        in1=stat[:, 4:5], op=AOP.subtract)
                nc.vector.tensor_scalar(out=stat[:, 4:5], in0=stat[:, 4:5],
                                        scalar1=float(EPS), scalar2=None,
                                        op0=AOP.add)            # var+eps
                nc.vector.reciprocal(stat[:, 5:6], stat[:, 4:5])
                nc.scalar.sqrt(stat[:, 6:7], stat[:, 5:6])      # rsqrt
                nc.vector.tensor_tensor(out=stat[:, 6:7],
                                        in0=gb_sb[:, 2 * i:2 * i + 1],
                                        in1=stat[:, 6:7], op=AOP.mult)
                nc.vector.tensor_tensor(out=stat[:, 7:8], in0=stat[:, 6:7],
                                        in1=stat[:, 2:3], op=AOP.mult)
                nc.vector.tensor_tensor(out=stat[:, 7:8],
                                        in0=gb_sb[:, 2 * i + 1:2 * i + 2],
                                        in1=stat[:, 7:8], op=AOP.subtract)

                # y = func(scale*agg + shift); x = y or x + y
                func = (mybir.ActivationFunctionType.Relu if i < L - 1
                        else mybir.ActivationFunctionType.Identity)
                for ch in range(n_chunks):
                    s = slice(ch * P, (ch + 1) * P)
                    if i == 0:
                        nc.scalar.activation(out=xT[:, s], in_=agg[:, s],
                                             func=func, bias=stat[:, 7:8],
                                             scale=stat[:, 6:7])
                    else:
                        yt = wk.tile([D, P], f32, tag="y")
                        nc.scalar.activation(out=yt[:], in_=agg[:, s],
                                             func=func, bias=stat[:, 7:8],
                                             scale=stat[:, 6:7])
                        nc.vector.tensor_tensor(out=xT[:, s], in0=xT[:, s],
                                                in1=yt[:], op=AOP.add)

            nc.sync.dma_start(out=out_ext[:, :], in_=xT[:])
    nc.compile()
    return nc


# ------------------------------------------------------------------ runner

class _Runner:
    """Persistent-jit PJRT runner (run_bass_via_pjrt, callable repeatedly)."""

    def __init__(self, nc, n_cores):
        import jax
        from jax.experimental.shard_map import shard_map
        from jax.sharding import Mesh, PartitionSpec
        from concourse import bass2jax
        self.jax = jax
        bass2jax.install_neuronx_cc_hook()
        in_names, out_names, out_avals, zero_outs = [], [], [], []
        partition_name = (nc.partition_id_tensor.name
                          if nc.partition_id_tensor else None)
        for alloc in nc.m.functions[0].allocations:
            if not isinstance(alloc, mybir.MemoryLocationSet):
                continue
            name = alloc.memorylocations[0].name
            if alloc.kind == "ExternalInput":
                if name != partition_name:
                    in_names.append(name)
            elif alloc.kind == "ExternalOutput":
                out_names.append(name)
                shape = tuple(alloc.tensor_shape)
                dtype = mybir.dt.np(alloc.dtype)
                out_avals.append(jax.core.ShapedArray(shape, dtype))
                zero_outs.append(np.zeros(shape, dtype))
        self.in_names, self.out_names = in_names, out_names
        self.out_avals, self.zero_outs = out_avals, zero_outs
        n_params, n_outs = len(in_names), len(out_avals)
        all_in = list(in_names) + list(out_names)
        if partition_name is not None:
            all_in.append(partition_name)
        from concourse.bass2jax import _bass_exec_p, partition_id_tensor

        def _body(*args):
            operands = list(args)
            if partition_name is not None:
                operands.append(partition_id_tensor())
            outs = _bass_exec_p.bind(
                *operands, out_avals=tuple(out_avals),
                in_names=tuple(all_in), out_names=tuple(out_names),
                lowering_input_output_aliases=(),
                sim_require_finite=False, sim_require_nnan=False, nc=nc)
            return tuple(outs)

        devices = jax.devices()[:n_cores]
        self.n_cores = n_cores
        self.mesh = Mesh(np.asarray(devices), ("core",))
        in_specs = (PartitionSpec("core"),) * (n_params + n_outs)
        out_specs = (PartitionSpec("core"),) * len(out_names)
        self.fn = jax.jit(
            shard_map(_body, mesh=self.mesh, in_specs=in_specs,
                      out_specs=out_specs, check_rep=False),
            keep_unused=True)
        self.dev_in = None

    def put(self, in_maps):
        from jax.sharding import NamedSharding, PartitionSpec
        sh = NamedSharding(self.mesh, PartitionSpec("core"))
        n = self.n_cores
        concat_in = [
            np.concatenate([np.asarray(in_maps[c][name]) for c in range(n)],
                           axis=0)
            for name in self.in_names]
        concat_zeros = [np.zeros((n * z.shape[0], *z.shape[1:]), z.dtype)
                        for z in self.zero_outs]
        self.dev_in = [self.jax.device_put(a, sh)
                       for a in concat_in + concat_zeros]
        self.jax.block_until_ready(self.dev_in)

    def __call__(self):
        out = self.fn(*self.dev_in)
        self.jax.block_until_ready(out)
        n = self.n_cores
        return [
            {name: np.asarray(out[i]).reshape(n, *self.out_avals[i].shape)[c]
             for i, name in enumerate(self.out_names)}
            for c in range(n)]


_CACHE = {}


def _get_runner(N, D, tiles_per_chunk, n_own, n_pad, n_chunks, NT):
    key = (N, D, NT)
    if key in _CACHE:
        return _CACHE[key]
    n_real_last = n_own - (n_chunks - 1) * P
    nc = _build(D, n_pad, n_chunks, NT, tiles_per_chunk, n_real_last, N)
    r = _Runner(nc, CORES)
    _CACHE[key] = r
    return r


def kernel(x, edge_index, Ws, bs, gammas, betas):
    x = np.asarray(x, np.float32)
    edge_index = np.asarray(edge_index, np.int32)
    Ws = np.asarray(Ws, np.float32)
    gammas = np.asarray(gammas, np.float32)
    betas = np.asarray(betas, np.float32)
    N, D = x.shape

    (xT, src_arr, rel_arr, tpc, n_own, n_pad, n_chunks, NT) = _prep(
        x, edge_index)
    r = _get_runner(N, D, tpc, n_own, n_pad, n_chunks, NT)

    Ws_flat = Ws.reshape(L * D, D)
    gb = np.zeros((D, 2 * L), np.float32)
    for i in range(L):
        gb[:, 2 * i] = gammas[i]
        gb[:, 2 * i + 1] = betas[i]

    in_maps = [{"xT_in": xT[c], "src_in": src_arr[c], "rel_in": rel_arr[c],
                "Ws_in": Ws_flat, "gb_in": gb} for c in range(CORES)]
    r.put(in_maps)
    res = r()
    out = np.empty((N, D), np.float32)
    for c in range(CORES):
        lo, hi = c * n_own, min((c + 1) * n_own, N)
        out[lo:hi] = res[c]["out"][:, :hi - lo].T
    return out


# revision 6
# speedup vs baseline: 680.6698x; 680.6698x over previous
"""CausalGNN forward on 8 Trainium2 NeuronCores (Bass/Tile).

Math (PyG-style GCN, 3 layers, BN training-mode, residuals):
    deg[v] = 1 + #{edges with dst=v};  dis = deg^-1/2
    per layer i:  h = x @ W_i;  agg[v] = sum_{e=(u,v)} dis_u dis_v h[u]
                  + dis_v^2 h[v]   (+ bias b_i, which BN cancels exactly)
                  y = BN(agg) (batch stats over all nodes), ReLU if i<2
                  x = y (i=0) or x + y (i>0)

Sharding: nodes (and the dst side of aggregation) are partitioned across 8
cores in contiguous ranges; edges live with their dst core, bucketed into
128-node chunks; self-edges are appended so the self term rides the same
path. The dis_u factor is folded into the gather table (h' = dis*h, exact:
row scaling commutes with x @ W), the dis_v factor is a per-column scale
applied once per chunk after PSUM accumulation.

Per layer, per core: h' for own nodes -> AllGather table -> per edge-tile:
indirect-gather h'[src] (128 rows), build a one-hot [edge, dst] on VectorE,
accumulate aggT[feature, dst] on TensorE in PSUM -> column scale + BN stats
-> AllReduce stats -> scale/shift (+ReLU) on ScalarE -> residual.

Everything on device except index bookkeeping: the host only buckets/sorts/
pads edge lists, transposes input/output layouts, and slices per-core
shards. Degrees, dis, norms, matmuls, BN are all computed on device.
"""
import sys
sys.path.insert(0, "/opt/trn_rl_repo")

import numpy as np

import concourse.bass as bass
import concourse.tile as tile
from concourse import bacc, mybir

f32 = mybir.dt.float32
i32 = mybir.dt.int32

P = 128
CORES = 8
L = 3
EPS = 1e-5


# ---------------------------------------------------------------- host prep

def _prep(x, edge_index):
    """Bucket edges by (core, chunk), append self-edges, pad to 128-tiles.

    Returns per-core arrays + the chunk tile counts (shared across cores).
    """
    N, D = x.shape
    E = edge_index.shape[1]
    n_own = (N + CORES - 1) // CORES            # nodes per core (last short)
    n_pad = ((n_own + P - 1) // P) * P          # padded to chunk multiple
    n_chunks = n_pad // P

    src = edge_index[0].astype(np.int64)
    dst = edge_index[1].astype(np.int64)

    # global padded-table row of node n (tables are [CORES*n_pad, D])
    def table_row(n):
        c = n // n_own
        return c * n_pad + (n - c * n_own)

    core_of = dst // n_own
    local = dst - core_of * n_own
    chunk_of = local // P
    dst_rel = local % P

    # self-edges: every real node, plus pad slots (src -> own row 0) so that
    # deg >= 1 everywhere and no inf/NaN enters the pipeline
    counts = np.zeros((CORES, n_chunks), np.int64)
    np.add.at(counts, (core_of, chunk_of), 1)
    counts += P  # one self-edge per slot in every chunk (incl. pad slots)

    tiles_per_chunk = ((counts.max(axis=0) + P - 1) // P).astype(np.int64)
    tile_base = np.concatenate([[0], np.cumsum(tiles_per_chunk)])
    NT = int(tile_base[-1])

    src_arr = np.zeros((CORES, P, NT), np.int32)      # table rows to gather
    rel_arr = np.full((CORES, P, NT), -1.0, np.float32)  # dst col or -1

    fill = np.zeros((CORES, n_chunks), np.int64)

    def put(c, ch, s_row, r):
        j = fill[c, ch]
        fill[c, ch] = j + 1
        t = tile_base[ch] + j // P
        p = j % P
        src_arr[c, p, t] = s_row
        rel_arr[c, p, t] = r

    # self-edges first (also covers pad slots)
    for c in range(CORES):
        base = c * n_own
        for ch in range(n_chunks):
            for r in range(P):
                n_local = ch * P + r
                if base + n_local < N and n_local < n_own:
                    put(c, ch, c * n_pad + n_local, r)
                else:
                    put(c, ch, c * n_pad, r)  # pad slot: gather own row 0
    # real edges (vectorized fill)
    order = np.lexsort((chunk_of, core_of))
    so, co, cho, dro = (src[order], core_of[order], chunk_of[order],
                        dst_rel[order])
    rows = table_row(so)
    grp = co * n_chunks + cho
    # positions within each (core, chunk) group, offset by current fill
    starts = np.searchsorted(grp, np.arange(CORES * n_chunks))
    pos = np.arange(E) - starts[grp] + fill.ravel()[grp]
    t_idx = tile_base[cho] + pos // P
    p_idx = pos % P
    src_arr[co, p_idx, t_idx] = rows
    rel_arr[co, p_idx, t_idx] = dro

    # per-core transposed, padded inputs
    xT = np.zeros((CORES, D, n_pad), np.float32)
    for c in range(CORES):
        lo, hi = c * n_own, min((c + 1) * n_own, N)
        xT[c, :, :hi - lo] = x[lo:hi].T
    return (xT, src_arr, rel_arr, tiles_per_chunk.astype(int), n_own, n_pad,
            n_chunks, NT)


# ------------------------------------------------------------- device build

def _build(D, n_pad, n_chunks, NT, tiles_per_chunk, n_real_last, N_total):
    """Build the SPMD Bass program (same for all cores)."""
    import os
    STAGE = int(os.environ.get("KERNEL_STAGE", "4"))
    nc = bacc.Bacc("TRN2", target_bir_lowering=False, debug=False,
                   num_devices=CORES)
    TBL = CORES * n_pad

    xT_in = nc.dram_tensor("xT_in", [D, n_pad], f32, kind="ExternalInput")
    src_in = nc.dram_tensor("src_in", [P, NT], i32, kind="ExternalInput")
    rel_in = nc.dram_tensor("rel_in", [P, NT], f32, kind="ExternalInput")
    Ws_in = nc.dram_tensor("Ws_in", [L * D, D], f32, kind="ExternalInput")
    gb_in = nc.dram_tensor("gb_in", [D, 2 * L], f32, kind="ExternalInput")
    out_ext = nc.dram_tensor("out", [D, n_pad], f32, kind="ExternalOutput")

    h_own = nc.dram_tensor("h_own", [n_pad, D], f32)
    h_tbl = nc.dram_tensor("h_tbl", [TBL, D], f32)
    h_gat = nc.dram_tensor("h_gat", [TBL, D], f32, kind="ExternalOutput")
    st_in = nc.dram_tensor("st_in", [P, 2], f32)
    st_out = nc.dram_tensor("st_out", [P, 2], f32)

    RG = [list(range(CORES))]
    AOP = mybir.AluOpType

    with tile.TileContext(nc) as tc:
        with tc.tile_pool(name="big", bufs=1) as big, \
             tc.tile_pool(name="sm", bufs=1) as sm, \
             tc.tile_pool(name="gat", bufs=12) as gat, \
             tc.tile_pool(name="oh", bufs=6) as ohp, \
             tc.tile_pool(name="work", bufs=3) as wk, \
             tc.tile_pool(name="ps", bufs=2, space="PSUM") as ps, \
             tc.tile_pool(name="psd", bufs=2, space="PSUM") as psd:

            # ---------------- persistent SBUF state
            xT = big.tile([D, n_pad], f32)
            nc.sync.dma_start(out=xT[:], in_=xT_in[:, :])
            src_sb = big.tile([P, NT], i32)
            nc.sync.dma_start(out=src_sb[:], in_=src_in[:, :])
            rel_sb = big.tile([P, NT], f32)
            nc.sync.dma_start(out=rel_sb[:], in_=rel_in[:, :])
            Ws_sb = sm.tile([D, L * D], f32)
            for i in range(L):
                nc.sync.dma_start(out=Ws_sb[:, i * D:(i + 1) * D],
                                  in_=Ws_in[i * D:(i + 1) * D, :])
            gb_sb = sm.tile([D, 2 * L], f32)
            nc.sync.dma_start(out=gb_sb[:], in_=gb_in[:, :])

            iota_i = sm.tile([P, P], i32)
            nc.gpsimd.iota(iota_i[:], pattern=[[1, P]], base=0,
                           channel_multiplier=0)
            iota_f = sm.tile([P, P], f32)
            nc.vector.tensor_copy(iota_f[:], iota_i[:])
            iota_col_i = sm.tile([P, P], i32)
            nc.gpsimd.iota(iota_col_i[:], pattern=[[1, P]], base=0,
                           channel_multiplier=1)
            iota_col = sm.tile([P, 1], f32)
            nc.vector.tensor_copy(iota_col[:], iota_col_i[:, 0:1])
            ones_col = sm.tile([P, 1], f32)
            nc.vector.memset(ones_col[:], 1.0)
            ones_sq = sm.tile([P, P], f32)
            nc.vector.memset(ones_sq[:], 1.0)

            dis_col = sm.tile([P, n_chunks], f32)   # dis, node-major cols
            dis_bc = big.tile([P, n_pad], f32)      # dis bcast over rows
            agg = big.tile([D, n_pad], f32)         # aggT per layer
            n_own_cols = (n_chunks - 1) * P + n_real_last
            if n_own_cols < n_pad:
                nc.vector.memset(agg[:, n_own_cols:], 0.0)
            slots = sm.tile([P, 2 * n_chunks], f32)  # per-chunk sums/sumsq
            stat = sm.tile([P, 8], f32)              # small scratch columns

            tb = np.concatenate([[0], np.cumsum(tiles_per_chunk)]).astype(int)

            # ---------------- one-time: degrees -> dis -> dis broadcast
            for ch in range(n_chunks):
                dps = psd.tile([P, 1], f32, space="PSUM", tag="deg")
                for t in range(tb[ch], tb[ch + 1]):
                    oht = ohp.tile([P, P], f32, tag="oh")
                    nc.vector.tensor_scalar(
                        out=oht[:], in0=iota_f[:],
                        scalar1=rel_sb[:, t:t + 1], scalar2=None,
                        op0=AOP.is_equal)
                    nc.tensor.matmul(out=dps[:], lhsT=oht[:], rhs=ones_col[:],
                                     start=(t == tb[ch]),
                                     stop=(t == tb[ch + 1] - 1))
                # dis = 1/sqrt(deg)
                nc.vector.reciprocal(stat[:, 0:1], dps[:])
                nc.scalar.sqrt(dis_col[:, ch:ch + 1], stat[:, 0:1])
                # dis broadcast to all partitions: ones128 @ diag(dis)
                diag = wk.tile([P, P], f32, tag="diag")
                nc.vector.tensor_scalar(
                    out=diag[:], in0=iota_f[:], scalar1=iota_col[:],
                    scalar2=dis_col[:, ch:ch + 1],
                    op0=AOP.is_equal, op1=AOP.mult)
                bps = psd.tile([P, P], f32, space="PSUM", tag="bc")
                nc.tensor.matmul(out=bps[:], lhsT=ones_sq[:], rhs=diag[:],
                                 start=True, stop=True)
                nc.scalar.copy(dis_bc[:, ch * P:(ch + 1) * P], bps[:])

            # ---------------- layers
            inv_n = 1.0 / float(N_total)
            if STAGE == 1:
                nc.scalar.copy(xT[:, 0:n_chunks], dis_col[:, 0:n_chunks])
            for i in range(range(0) and 0 or (L if STAGE >= 2 else 0)):
                # h' = dis * (x @ W_i), written row-major into own table rows
                for ch in range(n_chunks):
                    hps = ps.tile([P, D], f32, space="PSUM", tag="h")
                    nc.tensor.matmul(out=hps[:],
                                     lhsT=xT[:, ch * P:(ch + 1) * P],
                                     rhs=Ws_sb[:, i * D:(i + 1) * D],
                                     start=True, stop=True)
                    hsb = wk.tile([P, D], f32, tag="hsb")
                    nc.scalar.mul(out=hsb[:], in_=hps[:],
                                  mul=dis_col[:, ch:ch + 1])
                    nc.sync.dma_start(out=h_own[ch * P:(ch + 1) * P, :],
                                      in_=hsb[:])
                nc.gpsimd.collective_compute(
                    "AllGather", AOP.bypass, replica_groups=RG,
                    ins=[h_own[:, :]], outs=[h_tbl[:, :]])
                nc.sync.dma_start(out=h_gat[:, :], in_=h_tbl[:, :])
                if STAGE == 2:
                    continue

                # edge phase: gather + one-hot matmul, chunk accumulation
                for ch in range(n_chunks):
                    aps = ps.tile([D, P], f32, space="PSUM", tag="agg")
                    for t in range(tb[ch], tb[ch + 1]):
                        g = gat.tile([P, D], f32, tag="g")
                        nc.gpsimd.indirect_dma_start(
                            out=g[:], out_offset=None,
                            in_=h_gat[:, :],
                            in_offset=bass.IndirectOffsetOnAxis(
                                ap=src_sb[:, t:t + 1], axis=0))
                        oht = ohp.tile([P, P], f32, tag="oh")
                        nc.vector.tensor_scalar(
                            out=oht[:], in0=iota_f[:],
                            scalar1=rel_sb[:, t:t + 1], scalar2=None,
                            op0=AOP.is_equal)
                        nc.tensor.matmul(out=aps[:], lhsT=g[:], rhs=oht[:],
                                         start=(t == tb[ch]),
                                         stop=(t == tb[ch + 1] - 1))
                    # column scale by dis_dst; accumulate BN sums
                    w = P if ch < n_chunks - 1 else n_real_last
                    nc.vector.tensor_tensor(
                        out=agg[:, ch * P:ch * P + w],
                        in0=aps[:, 0:w],
                        in1=dis_bc[:, ch * P:ch * P + w],
                        op=AOP.mult)
                    nc.vector.tensor_reduce(
                        out=slots[:, ch:ch + 1],
                        in_=agg[:, ch * P:ch * P + w],
                        axis=mybir.AxisListType.X, op=AOP.add)
                    sq = wk.tile([P, P], f32, tag="sq")
                    nc.vector.tensor_tensor(
                        out=sq[:, 0:w], in0=agg[:, ch * P:ch * P + w],
                        in1=agg[:, ch * P:ch * P + w], op=AOP.mult)
                    nc.vector.tensor_reduce(
                        out=slots[:, n_chunks + ch:n_chunks + ch + 1],
                        in_=sq[:, 0:w],
                        axis=mybir.AxisListType.X, op=AOP.add)

                if STAGE == 3:
                    for ch in range(n_chunks):
                        s = slice(ch * P, (ch + 1) * P)
                        nc.vector.tensor_copy(xT[:, s], agg[:, s])
                    continue
                # stats: reduce chunk slots, AllReduce, scale/shift
                nc.vector.tensor_reduce(
                    out=stat[:, 0:1], in_=slots[:, 0:n_chunks],
                    axis=mybir.AxisListType.X, op=AOP.add)
                nc.vector.tensor_reduce(
                    out=stat[:, 1:2], in_=slots[:, n_chunks:2 * n_chunks],
                    axis=mybir.AxisListType.X, op=AOP.add)
                sin = wk.tile([P, 2], f32, tag="stin")
                nc.vector.tensor_copy(sin[:], stat[:, 0:2])
                nc.sync.dma_start(out=st_in[:, :], in_=sin[:])
                nc.gpsimd.collective_compute(
                    "AllReduce", AOP.add, replica_groups=RG,
                    ins=[st_in[:, :]], outs=[st_out[:, :]])
                sout = wk.tile([P, 2], f32, tag="stout")
                nc.sync.dma_start(out=sout[:], in_=st_out[:, :])
                # mean, var, scale = gamma*rsqrt(var+eps), shift = beta-sc*mean
                nc.vector.tensor_scalar(out=stat[:, 2:3], in0=sout[:, 0:1],
                                        scalar1=inv_n, scalar2=None,
                                        op0=AOP.mult)           # mean
                nc.vector.tensor_scalar(out=stat[:, 3:4], in0=sout[:, 1:2],
                                        scalar1=inv_n, scalar2=None,
                                        op0=AOP.mult)           # E[x^2]
                nc.vector.tensor_tensor(out=stat[:, 4:5], in0=stat[:, 2:3],
                                        in1=stat[:, 2:3], op=AOP.mult)
                nc.vector.tensor_tensor(out=stat[:, 4:5], in0=stat[:, 3:4],
                                        in1=stat[:, 4:5], op=AOP.subtract)
                nc.vector.tensor_scalar(out=stat[:, 4:5], in0=stat[:, 4:5],
                                        scalar1=float(EPS), scalar2=None,
                                        op0=AOP.add)            # var+eps
                nc.vector.reciprocal(stat[:, 5:6], stat[:, 4:5])
                nc.scalar.sqrt(stat[:, 6:7], stat[:, 5:6])      # rsqrt
                nc.vector.tensor_tensor(out=stat[:, 6:7],
                                        in0=gb_sb[:, 2 * i:2 * i + 1],
                                        in1=stat[:, 6:7], op=AOP.mult)
                nc.vector.tensor_tensor(out=stat[:, 7:8], in0=stat[:, 6:7],
                                        in1=stat[:, 2:3], op=AOP.mult)
                nc.vector.tensor_tensor(out=stat[:, 7:8],
                                        in0=gb_sb[:, 2 * i + 1:2 * i + 2],
                                        in1=stat[:, 7:8], op=AOP.subtract)

                # y = func(scale*agg + shift); x = y or x + y
                func = (mybir.ActivationFunctionType.Relu if i < L - 1
                        else mybir.ActivationFunctionType.Identity)
                for ch in range(n_chunks):
                    s = slice(ch * P, (ch + 1) * P)
                    if i == 0:
                        nc.scalar.activation(out=xT[:, s], in_=agg[:, s],
                                             func=func, bias=stat[:, 7:8],
                                             scale=stat[:, 6:7])
                    else:
                        yt = wk.tile([D, P], f32, tag="y")
                        nc.scalar.activation(out=yt[:], in_=agg[:, s],
                                             func=func, bias=stat[:, 7:8],
                                             scale=stat[:, 6:7])
                        nc.vector.tensor_tensor(out=xT[:, s], in0=xT[:, s],
                                                in1=yt[:], op=AOP.add)

            nc.sync.dma_start(out=out_ext[:, :], in_=xT[:])
    nc.compile()
    return nc


# ------------------------------------------------------------------ runner

class _Runner:
    """Persistent-jit PJRT runner (run_bass_via_pjrt, callable repeatedly)."""

    def __init__(self, nc, n_cores):
        import jax
        from jax.experimental.shard_map import shard_map
        from jax.sharding import Mesh, PartitionSpec
        from concourse import bass2jax
        self.jax = jax
        bass2jax.install_neuronx_cc_hook()
        in_names, out_names, out_avals, zero_outs = [], [], [], []
        partition_name = (nc.partition_id_tensor.name
                          if nc.partition_id_tensor else None)
        for alloc in nc.m.functions[0].allocations:
            if not isinstance(alloc, mybir.MemoryLocationSet):
                continue
            name = alloc.memorylocations[0].name
            if alloc.kind == "ExternalInput":
                if name != partition_name:
                    in_names.append(name)
            elif alloc.kind == "ExternalOutput":
                out_names.append(name)
                shape = tuple(alloc.tensor_shape)
                dtype = mybir.dt.np(alloc.dtype)
                out_avals.append(jax.core.ShapedArray(shape, dtype))
                zero_outs.append(np.zeros(shape, dtype))
        self.in_names, self.out_names = in_names, out_names
        self.out_avals, self.zero_outs = out_avals, zero_outs
        n_params, n_outs = len(in_names), len(out_avals)
        all_in = list(in_names) + list(out_names)
        if partition_name is not None:
            all_in.append(partition_name)
        from concourse.bass2jax import _bass_exec_p, partition_id_tensor

        def _body(*args):
            operands = list(args)
            if partition_name is not None:
                operands.append(partition_id_tensor())
            outs = _bass_exec_p.bind(
                *operands, out_avals=tuple(out_avals),
                in_names=tuple(all_in), out_names=tuple(out_names),
                lowering_input_output_aliases=(),
                sim_require_finite=False, sim_require_nnan=False, nc=nc)
            return tuple(outs)

        devices = jax.devices()[:n_cores]
        self.n_cores = n_cores
        self.mesh = Mesh(np.asarray(devices), ("core",))
        in_specs = (PartitionSpec("core"),) * (n_params + n_outs)
        out_specs = (PartitionSpec("core"),) * len(out_names)
        self.fn = jax.jit(
            shard_map(_body, mesh=self.mesh, in_specs=in_specs,
                      out_specs=out_specs, check_rep=False),
            keep_unused=True)
        self.dev_in = None

    def put(self, in_maps):
        from jax.sharding import NamedSharding, PartitionSpec
        sh = NamedSharding(self.mesh, PartitionSpec("core"))
        n = self.n_cores
        concat_in = [
            np.concatenate([np.asarray(in_maps[c][name]) for c in range(n)],
                           axis=0)
            for name in self.in_names]
        concat_zeros = [np.zeros((n * z.shape[0], *z.shape[1:]), z.dtype)
                        for z in self.zero_outs]
        self.dev_in = [self.jax.device_put(a, sh)
                       for a in concat_in + concat_zeros]
        self.jax.block_until_ready(self.dev_in)

    def __call__(self, fetch=("out",)):
        out = self.fn(*self.dev_in)
        self.jax.block_until_ready(out)
        n = self.n_cores
        return [
            {name: np.asarray(out[i]).reshape(n, *self.out_avals[i].shape)[c]
             for i, name in enumerate(self.out_names) if name in fetch}
            for c in range(n)]


_CACHE = {}


def _get_runner(N, D, tiles_per_chunk, n_own, n_pad, n_chunks, NT):
    key = (N, D, NT)
    if key in _CACHE:
        return _CACHE[key]
    n_real_last = n_own - (n_chunks - 1) * P
    nc = _build(D, n_pad, n_chunks, NT, tiles_per_chunk, n_real_last, N)
    r = _Runner(nc, CORES)
    _CACHE[key] = r
    return r


def kernel(x, edge_index, Ws, bs, gammas, betas):
    x = np.asarray(x, np.float32)
    edge_index = np.asarray(edge_index, np.int32)
    Ws = np.asarray(Ws, np.float32)
    gammas = np.asarray(gammas, np.float32)
    betas = np.asarray(betas, np.float32)
    N, D = x.shape

    (xT, src_arr, rel_arr, tpc, n_own, n_pad, n_chunks, NT) = _prep(
        x, edge_index)
    r = _get_runner(N, D, tpc, n_own, n_pad, n_chunks, NT)

    Ws_flat = Ws.reshape(L * D, D)
    gb = np.zeros((D, 2 * L), np.float32)
    for i in range(L):
        gb[:, 2 * i] = gammas[i]
        gb[:, 2 * i + 1] = betas[i]

    in_maps = [{"xT_in": xT[c], "src_in": src_arr[c], "rel_in": rel_arr[c],
                "Ws_in": Ws_flat, "gb_in": gb} for c in range(CORES)]
    r.put(in_maps)
    res = r()
    out = np.empty((N, D), np.float32)
    for c in range(CORES):
        lo, hi = c * n_own, min((c + 1) * n_own, N)
        out[lo:hi] = res[c]["out"][:, :hi - lo].T
    return out


# revision 8
# speedup vs baseline: 1543.5344x; 2.2677x over previous
"""CausalGNN forward on 8 Trainium2 NeuronCores (Bass/Tile).

Math (PyG-style GCN, 3 layers, BN training-mode, residuals):
    deg[v] = 1 + #{edges with dst=v};  dis = deg^-1/2
    per layer i:  h = x @ W_i;  agg[v] = sum_{e=(u,v)} dis_u dis_v h[u]
                  + dis_v^2 h[v]   (+ bias b_i, which BN cancels exactly)
                  y = BN(agg) (batch stats over all nodes), ReLU if i<2
                  x = y (i=0) or x + y (i>0)

Sharding: nodes (and the dst side of aggregation) are partitioned across 8
cores in contiguous ranges; edges live with their dst core, bucketed into
128-node chunks; self-edges are appended so the self term rides the same
path. The dis_u factor is folded into the gather table (h' = dis*h, exact:
row scaling commutes with x @ W), the dis_v factor is a per-column scale
applied once per chunk after PSUM accumulation.

Per layer, per core: h' for own nodes -> AllGather table -> per edge-tile:
indirect-gather h'[src] (128 rows), build a one-hot [edge, dst] on VectorE,
accumulate aggT[feature, dst] on TensorE in PSUM -> column scale + BN stats
-> AllReduce stats -> scale/shift (+ReLU) on ScalarE -> residual.

Everything on device except index bookkeeping: the host only buckets/sorts/
pads edge lists, transposes input/output layouts, and slices per-core
shards. Degrees, dis, norms, matmuls, BN are all computed on device.
"""
import sys
sys.path.insert(0, "/opt/trn_rl_repo")

import numpy as np

import concourse.bass as bass
import concourse.tile as tile
from concourse import bacc, mybir

f32 = mybir.dt.float32
i32 = mybir.dt.int32

P = 128
CORES = 8
L = 3
EPS = 1e-5


# ---------------------------------------------------------------- host prep

def _prep(x, edge_index):
    """Bucket edges by (core, chunk), append self-edges, pad to 128-tiles.

    Returns per-core arrays + the chunk tile counts (shared across cores).
    """
    N, D = x.shape
    E = edge_index.shape[1]
    n_own = (N + CORES - 1) // CORES            # nodes per core (last short)
    n_pad = ((n_own + P - 1) // P) * P          # padded to chunk multiple
    n_chunks = n_pad // P

    src = edge_index[0].astype(np.int64)
    dst = edge_index[1].astype(np.int64)

    # global padded-table row of node n (tables are [CORES*n_pad, D])
    def table_row(n):
        c = n // n_own
        return c * n_pad + (n - c * n_own)

    core_of = dst // n_own
    local = dst - core_of * n_own
    chunk_of = local // P
    dst_rel = local % P

    # self-edges: every real node, plus pad slots (src -> own row 0) so that
    # deg >= 1 everywhere and no inf/NaN enters the pipeline
    counts = np.zeros((CORES, n_chunks), np.int64)
    np.add.at(counts, (core_of, chunk_of), 1)
    counts += P  # one self-edge per slot in every chunk (incl. pad slots)

    tiles_per_chunk = ((counts.max(axis=0) + P - 1) // P).astype(np.int64)
    tile_base = np.concatenate([[0], np.cumsum(tiles_per_chunk)])
    NT = int(tile_base[-1])

    src_arr = np.zeros((CORES, P, NT), np.int32)      # table rows to gather
    rel_arr = np.full((CORES, P, NT), -1.0, np.float32)  # dst col or -1

    fill = np.zeros((CORES, n_chunks), np.int64)

    def put(c, ch, s_row, r):
        j = fill[c, ch]
        fill[c, ch] = j + 1
        t = tile_base[ch] + j // P
        p = j % P
        src_arr[c, p, t] = s_row
        rel_arr[c, p, t] = r

    # self-edges first (also covers pad slots)
    for c in range(CORES):
        base = c * n_own
        for ch in range(n_chunks):
            for r in range(P):
                n_local = ch * P + r
                if base + n_local < N and n_local < n_own:
                    put(c, ch, c * n_pad + n_local, r)
                else:
                    put(c, ch, c * n_pad, r)  # pad slot: gather own row 0
    # real edges (vectorized fill)
    order = np.lexsort((chunk_of, core_of))
    so, co, cho, dro = (src[order], core_of[order], chunk_of[order],
                        dst_rel[order])
    rows = table_row(so)
    grp = co * n_chunks + cho
    # positions within each (core, chunk) group, offset by current fill
    starts = np.searchsorted(grp, np.arange(CORES * n_chunks))
    pos = np.arange(E) - starts[grp] + fill.ravel()[grp]
    t_idx = tile_base[cho] + pos // P
    p_idx = pos % P
    src_arr[co, p_idx, t_idx] = rows
    rel_arr[co, p_idx, t_idx] = dro

    # per-core transposed, padded inputs
    xT = np.zeros((CORES, D, n_pad), np.float32)
    for c in range(CORES):
        lo, hi = c * n_own, min((c + 1) * n_own, N)
        xT[c, :, :hi - lo] = x[lo:hi].T
    return (xT, src_arr, rel_arr, tiles_per_chunk.astype(int), n_own, n_pad,
            n_chunks, NT)


# ------------------------------------------------------------- device build

def _build(D, n_pad, n_chunks, NT, tiles_per_chunk, n_real_last, N_total):
    """Build the SPMD Bass program (same for all cores)."""
    import os
    STAGE = int(os.environ.get("KERNEL_STAGE", "4"))
    nc = bacc.Bacc("TRN2", target_bir_lowering=False, debug=False,
                   num_devices=CORES)
    TBL = CORES * n_pad

    xT_in = nc.dram_tensor("xT_in", [D, n_pad], f32, kind="ExternalInput")
    src_in = nc.dram_tensor("src_in", [P, NT], i32, kind="ExternalInput")
    rel_in = nc.dram_tensor("rel_in", [P, NT], f32, kind="ExternalInput")
    Ws_in = nc.dram_tensor("Ws_in", [L * D, D], f32, kind="ExternalInput")
    gb_in = nc.dram_tensor("gb_in", [D, 2 * L], f32, kind="ExternalInput")
    out_ext = nc.dram_tensor("out", [D, n_pad], f32, kind="ExternalOutput")

    h_own = nc.dram_tensor("h_own", [n_pad, D], f32)
    h_tbl = nc.dram_tensor("h_tbl", [TBL, D], f32)
    h_gat = nc.dram_tensor("h_gat", [TBL, D], f32, kind="ExternalOutput")
    st_in = nc.dram_tensor("st_in", [P, 2], f32)
    st_out = nc.dram_tensor("st_out", [P, 2], f32)

    RG = [list(range(CORES))]
    AOP = mybir.AluOpType

    with tile.TileContext(nc) as tc:
        with tc.tile_pool(name="big", bufs=1) as big, \
             tc.tile_pool(name="sm", bufs=1) as sm, \
             tc.tile_pool(name="gat", bufs=12) as gat, \
             tc.tile_pool(name="oh", bufs=6) as ohp, \
             tc.tile_pool(name="work", bufs=3) as wk, \
             tc.tile_pool(name="ps", bufs=2, space="PSUM") as ps, \
             tc.tile_pool(name="psd", bufs=2, space="PSUM") as psd:

            # ---------------- persistent SBUF state
            xT = big.tile([D, n_pad], f32)
            nc.sync.dma_start(out=xT[:], in_=xT_in[:, :])
            src_sb = big.tile([P, NT], i32)
            nc.sync.dma_start(out=src_sb[:], in_=src_in[:, :])
            rel_sb = big.tile([P, NT], f32)
            nc.sync.dma_start(out=rel_sb[:], in_=rel_in[:, :])
            Ws_sb = sm.tile([D, L * D], f32)
            for i in range(L):
                nc.sync.dma_start(out=Ws_sb[:, i * D:(i + 1) * D],
                                  in_=Ws_in[i * D:(i + 1) * D, :])
            gb_sb = sm.tile([D, 2 * L], f32)
            nc.sync.dma_start(out=gb_sb[:], in_=gb_in[:, :])

            iota_i = sm.tile([P, P], i32)
            nc.gpsimd.iota(iota_i[:], pattern=[[1, P]], base=0,
                           channel_multiplier=0)
            iota_f = sm.tile([P, P], f32)
            nc.vector.tensor_copy(iota_f[:], iota_i[:])
            iota_col_i = sm.tile([P, P], i32)
            nc.gpsimd.iota(iota_col_i[:], pattern=[[1, P]], base=0,
                           channel_multiplier=1)
            iota_col = sm.tile([P, 1], f32)
            nc.vector.tensor_copy(iota_col[:], iota_col_i[:, 0:1])
            ones_col = sm.tile([P, 1], f32)
            nc.vector.memset(ones_col[:], 1.0)
            ones_sq = sm.tile([P, P], f32)
            nc.vector.memset(ones_sq[:], 1.0)

            dis_col = sm.tile([P, n_chunks], f32)   # dis, node-major cols
            dis_bc = big.tile([P, n_pad], f32)      # dis bcast over rows
            agg = big.tile([D, n_pad], f32)         # aggT per layer
            n_own_cols = (n_chunks - 1) * P + n_real_last
            if n_own_cols < n_pad:
                nc.vector.memset(agg[:, n_own_cols:], 0.0)
            slots = sm.tile([P, 2 * n_chunks], f32)  # per-chunk sums/sumsq
            stat = sm.tile([P, 8], f32)              # small scratch columns

            tb = np.concatenate([[0], np.cumsum(tiles_per_chunk)]).astype(int)

            # ---------------- one-time: degrees -> dis -> dis broadcast
            for ch in range(n_chunks):
                dps = psd.tile([P, 1], f32, space="PSUM", tag="deg")
                for t in range(tb[ch], tb[ch + 1]):
                    oht = ohp.tile([P, P], f32, tag="oh")
                    nc.vector.tensor_scalar(
                        out=oht[:], in0=iota_f[:],
                        scalar1=rel_sb[:, t:t + 1], scalar2=None,
                        op0=AOP.is_equal)
                    nc.tensor.matmul(out=dps[:], lhsT=oht[:], rhs=ones_col[:],
                                     start=(t == tb[ch]),
                                     stop=(t == tb[ch + 1] - 1))
                # dis = 1/sqrt(deg)
                nc.vector.reciprocal(stat[:, 0:1], dps[:])
                nc.scalar.sqrt(dis_col[:, ch:ch + 1], stat[:, 0:1])
                # dis broadcast to all partitions: ones128 @ diag(dis)
                diag = wk.tile([P, P], f32, tag="diag")
                nc.vector.tensor_scalar(
                    out=diag[:], in0=iota_f[:], scalar1=iota_col[:],
                    scalar2=dis_col[:, ch:ch + 1],
                    op0=AOP.is_equal, op1=AOP.mult)
                bps = psd.tile([P, P], f32, space="PSUM", tag="bc")
                nc.tensor.matmul(out=bps[:], lhsT=ones_sq[:], rhs=diag[:],
                                 start=True, stop=True)
                nc.scalar.copy(dis_bc[:, ch * P:(ch + 1) * P], bps[:])

            # ---------------- layers
            inv_n = 1.0 / float(N_total)
            if STAGE == 1:
                nc.scalar.copy(xT[:, 0:n_chunks], dis_col[:, 0:n_chunks])
            for i in range(range(0) and 0 or (L if STAGE >= 2 else 0)):
                # h' = dis * (x @ W_i), written row-major into own table rows
                for ch in range(n_chunks):
                    hps = ps.tile([P, D], f32, space="PSUM", tag="h")
                    nc.tensor.matmul(out=hps[:],
                                     lhsT=xT[:, ch * P:(ch + 1) * P],
                                     rhs=Ws_sb[:, i * D:(i + 1) * D],
                                     start=True, stop=True)
                    hsb = wk.tile([P, D], f32, tag="hsb")
                    nc.scalar.mul(out=hsb[:], in_=hps[:],
                                  mul=dis_col[:, ch:ch + 1])
                    nc.sync.dma_start(out=h_own[ch * P:(ch + 1) * P, :],
                                      in_=hsb[:])
                nc.gpsimd.collective_compute(
                    "AllGather", AOP.bypass, replica_groups=RG,
                    ins=[h_own[:, :]], outs=[h_tbl[:, :]])
                nc.sync.dma_start(out=h_gat[:, :], in_=h_tbl[:, :])
                if STAGE == 2:
                    continue

                # edge phase: gather + one-hot matmul, chunk accumulation
                for ch in range(n_chunks):
                    aps = ps.tile([D, P], f32, space="PSUM", tag="agg")
                    for t in range(tb[ch], tb[ch + 1]):
                        g = gat.tile([P, D], f32, tag="g")
                        nc.gpsimd.indirect_dma_start(
                            out=g[:], out_offset=None,
                            in_=h_gat[:, :],
                            in_offset=bass.IndirectOffsetOnAxis(
                                ap=src_sb[:, t:t + 1], axis=0))
                        oht = ohp.tile([P, P], f32, tag="oh")
                        nc.vector.tensor_scalar(
                            out=oht[:], in0=iota_f[:],
                            scalar1=rel_sb[:, t:t + 1], scalar2=None,
                            op0=AOP.is_equal)
                        nc.tensor.matmul(out=aps[:], lhsT=g[:], rhs=oht[:],
                                         start=(t == tb[ch]),
                                         stop=(t == tb[ch + 1] - 1))
                    # column scale by dis_dst; accumulate BN sums
                    w = P if ch < n_chunks - 1 else n_real_last
                    nc.vector.tensor_tensor(
                        out=agg[:, ch * P:ch * P + w],
                        in0=aps[:, 0:w],
                        in1=dis_bc[:, ch * P:ch * P + w],
                        op=AOP.mult)
                    nc.vector.tensor_reduce(
                        out=slots[:, ch:ch + 1],
                        in_=agg[:, ch * P:ch * P + w],
                        axis=mybir.AxisListType.X, op=AOP.add)
                    sq = wk.tile([P, P], f32, tag="sq")
                    nc.vector.tensor_tensor(
                        out=sq[:, 0:w], in0=agg[:, ch * P:ch * P + w],
                        in1=agg[:, ch * P:ch * P + w], op=AOP.mult)
                    nc.vector.tensor_reduce(
                        out=slots[:, n_chunks + ch:n_chunks + ch + 1],
                        in_=sq[:, 0:w],
                        axis=mybir.AxisListType.X, op=AOP.add)

                if STAGE == 3:
                    for ch in range(n_chunks):
                        s = slice(ch * P, (ch + 1) * P)
                        nc.vector.tensor_copy(xT[:, s], agg[:, s])
                    continue
                # stats: reduce chunk slots, AllReduce, scale/shift
                nc.vector.tensor_reduce(
                    out=stat[:, 0:1], in_=slots[:, 0:n_chunks],
                    axis=mybir.AxisListType.X, op=AOP.add)
                nc.vector.tensor_reduce(
                    out=stat[:, 1:2], in_=slots[:, n_chunks:2 * n_chunks],
                    axis=mybir.AxisListType.X, op=AOP.add)
                sin = wk.tile([P, 2], f32, tag="stin")
                nc.vector.tensor_copy(sin[:], stat[:, 0:2])
                nc.sync.dma_start(out=st_in[:, :], in_=sin[:])
                nc.gpsimd.collective_compute(
                    "AllReduce", AOP.add, replica_groups=RG,
                    ins=[st_in[:, :]], outs=[st_out[:, :]])
                sout = wk.tile([P, 2], f32, tag="stout")
                nc.sync.dma_start(out=sout[:], in_=st_out[:, :])
                # mean, var, scale = gamma*rsqrt(var+eps), shift = beta-sc*mean
                nc.vector.tensor_scalar(out=stat[:, 2:3], in0=sout[:, 0:1],
                                        scalar1=inv_n, scalar2=None,
                                        op0=AOP.mult)           # mean
                nc.vector.tensor_scalar(out=stat[:, 3:4], in0=sout[:, 1:2],
                                        scalar1=inv_n, scalar2=None,
                                        op0=AOP.mult)           # E[x^2]
                nc.vector.tensor_tensor(out=stat[:, 4:5], in0=stat[:, 2:3],
                                        in1=stat[:, 2:3], op=AOP.mult)
                nc.vector.tensor_tensor(out=stat[:, 4:5], in0=stat[:, 3:4],
                                        in1=stat[:, 4:5], op=AOP.subtract)
                nc.vector.tensor_scalar(out=stat[:, 4:5], in0=stat[:, 4:5],
                                        scalar1=float(EPS), scalar2=None,
                                        op0=AOP.add)            # var+eps
                nc.vector.reciprocal(stat[:, 5:6], stat[:, 4:5])
                nc.scalar.sqrt(stat[:, 6:7], stat[:, 5:6])      # rsqrt
                nc.vector.tensor_tensor(out=stat[:, 6:7],
                                        in0=gb_sb[:, 2 * i:2 * i + 1],
                                        in1=stat[:, 6:7], op=AOP.mult)
                nc.vector.tensor_tensor(out=stat[:, 7:8], in0=stat[:, 6:7],
                                        in1=stat[:, 2:3], op=AOP.mult)
                nc.vector.tensor_tensor(out=stat[:, 7:8],
                                        in0=gb_sb[:, 2 * i + 1:2 * i + 2],
                                        in1=stat[:, 7:8], op=AOP.subtract)

                # y = func(scale*agg + shift); x = y or x + y
                func = (mybir.ActivationFunctionType.Relu if i < L - 1
                        else mybir.ActivationFunctionType.Identity)
                for ch in range(n_chunks):
                    s = slice(ch * P, (ch + 1) * P)
                    if i == 0:
                        nc.scalar.activation(out=xT[:, s], in_=agg[:, s],
                                             func=func, bias=stat[:, 7:8],
                                             scale=stat[:, 6:7])
                    else:
                        yt = wk.tile([D, P], f32, tag="y")
                        nc.scalar.activation(out=yt[:], in_=agg[:, s],
                                             func=func, bias=stat[:, 7:8],
                                             scale=stat[:, 6:7])
                        nc.vector.tensor_tensor(out=xT[:, s], in0=xT[:, s],
                                                in1=yt[:], op=AOP.add)

            nc.sync.dma_start(out=out_ext[:, :], in_=xT[:])
    nc.compile()
    return nc


# ------------------------------------------------------------------ runner

class _Runner:
    """Persistent-jit PJRT runner (run_bass_via_pjrt, callable repeatedly)."""

    def __init__(self, nc, n_cores):
        import jax
        from jax.experimental.shard_map import shard_map
        from jax.sharding import Mesh, PartitionSpec
        from concourse import bass2jax
        self.jax = jax
        bass2jax.install_neuronx_cc_hook()
        in_names, out_names, out_avals, zero_outs = [], [], [], []
        partition_name = (nc.partition_id_tensor.name
                          if nc.partition_id_tensor else None)
        for alloc in nc.m.functions[0].allocations:
            if not isinstance(alloc, mybir.MemoryLocationSet):
                continue
            name = alloc.memorylocations[0].name
            if alloc.kind == "ExternalInput":
                if name != partition_name:
                    in_names.append(name)
            elif alloc.kind == "ExternalOutput":
                out_names.append(name)
                shape = tuple(alloc.tensor_shape)
                dtype = mybir.dt.np(alloc.dtype)
                out_avals.append(jax.core.ShapedArray(shape, dtype))
                zero_outs.append(np.zeros(shape, dtype))
        self.in_names, self.out_names = in_names, out_names
        self.out_avals, self.zero_outs = out_avals, zero_outs
        n_params, n_outs = len(in_names), len(out_avals)
        all_in = list(in_names) + list(out_names)
        if partition_name is not None:
            all_in.append(partition_name)
        from concourse.bass2jax import _bass_exec_p, partition_id_tensor

        def _body(*args):
            operands = list(args)
            if partition_name is not None:
                operands.append(partition_id_tensor())
            outs = _bass_exec_p.bind(
                *operands, out_avals=tuple(out_avals),
                in_names=tuple(all_in), out_names=tuple(out_names),
                lowering_input_output_aliases=(),
                sim_require_finite=False, sim_require_nnan=False, nc=nc)
            return tuple(outs)

        devices = jax.devices()[:n_cores]
        self.n_cores = n_cores
        self.mesh = Mesh(np.asarray(devices), ("core",))
        in_specs = (PartitionSpec("core"),) * (n_params + n_outs)
        out_specs = (PartitionSpec("core"),) * len(out_names)
        self.fn = jax.jit(
            shard_map(_body, mesh=self.mesh, in_specs=in_specs,
                      out_specs=out_specs, check_rep=False),
            keep_unused=True)
        self.dev_in = None

    def put(self, in_maps):
        from jax.sharding import NamedSharding, PartitionSpec
        sh = NamedSharding(self.mesh, PartitionSpec("core"))
        n = self.n_cores
        concat_in = [
            np.concatenate([np.asarray(in_maps[c][name]) for c in range(n)],
                           axis=0)
            for name in self.in_names]
        concat_zeros = [np.zeros((n * z.shape[0], *z.shape[1:]), z.dtype)
                        for z in self.zero_outs]
        self.dev_in = [self.jax.device_put(a, sh)
                       for a in concat_in + concat_zeros]
        self.jax.block_until_ready(self.dev_in)

    def __call__(self, fetch=("out",)):
        out = self.fn(*self.dev_in)
        self.jax.block_until_ready(out)
        n = self.n_cores
        return [
            {name: np.asarray(out[i]).reshape(n, *self.out_avals[i].shape)[c]
             for i, name in enumerate(self.out_names) if name in fetch}
            for c in range(n)]


_CACHE = {}


def _get_runner(N, D, tiles_per_chunk, n_own, n_pad, n_chunks, NT):
    key = (N, D, NT)
    if key in _CACHE:
        return _CACHE[key]
    n_real_last = n_own - (n_chunks - 1) * P
    nc = _build(D, n_pad, n_chunks, NT, tiles_per_chunk, n_real_last, N)
    r = _Runner(nc, CORES)
    _CACHE[key] = r
    return r


def kernel(x, edge_index, Ws, bs, gammas, betas):
    x = np.asarray(x, np.float32)
    edge_index = np.asarray(edge_index, np.int32)
    Ws = np.asarray(Ws, np.float32)
    gammas = np.asarray(gammas, np.float32)
    betas = np.asarray(betas, np.float32)
    N, D = x.shape

    (xT, src_arr, rel_arr, tpc, n_own, n_pad, n_chunks, NT) = _prep(
        x, edge_index)
    r = _get_runner(N, D, tpc, n_own, n_pad, n_chunks, NT)

    Ws_flat = Ws.reshape(L * D, D)
    gb = np.zeros((D, 2 * L), np.float32)
    for i in range(L):
        gb[:, 2 * i] = gammas[i]
        gb[:, 2 * i + 1] = betas[i]

    in_maps = [{"xT_in": xT[c], "src_in": src_arr[c], "rel_in": rel_arr[c],
                "Ws_in": Ws_flat, "gb_in": gb} for c in range(CORES)]
    r.put(in_maps)
    res = r()
    out = np.empty((N, D), np.float32)
    for c in range(CORES):
        lo, hi = c * n_own, min((c + 1) * n_own, N)
        out[lo:hi] = res[c]["out"][:, :hi - lo].T
    return out


# revision 9
# speedup vs baseline: 1857.9357x; 1.2037x over previous
"""CausalGNN forward on 8 Trainium2 NeuronCores (Bass/Tile).

Math (PyG-style GCN, 3 layers, BN training-mode, residuals):
    deg[v] = 1 + #{edges with dst=v};  dis = deg^-1/2
    per layer i:  h = x @ W_i;  agg[v] = sum_{e=(u,v)} dis_u dis_v h[u]
                  + dis_v^2 h[v]   (+ bias b_i, which BN cancels exactly)
                  y = BN(agg) (batch stats over all nodes), ReLU if i<2
                  x = y (i=0) or x + y (i>0)

Sharding: nodes (and the dst side of aggregation) are partitioned across 8
cores in contiguous ranges; edges live with their dst core, bucketed into
128-node chunks; self-edges are appended so the self term rides the same
path. The dis_u factor is folded into the gather table (h' = dis*h, exact:
row scaling commutes with x @ W), the dis_v factor is a per-column scale
applied once per chunk after PSUM accumulation.

Per layer, per core: h' for own nodes -> AllGather table -> per edge-tile:
indirect-gather h'[src] (128 rows), build a one-hot [edge, dst] on VectorE,
accumulate aggT[feature, dst] on TensorE in PSUM -> column scale + BN stats
-> AllReduce stats -> scale/shift (+ReLU) on ScalarE -> residual.

Everything on device except index bookkeeping: the host only buckets/sorts/
pads edge lists, transposes input/output layouts, and slices per-core
shards. Degrees, dis, norms, matmuls, BN are all computed on device.
"""
import sys
sys.path.insert(0, "/opt/trn_rl_repo")

import numpy as np

import concourse.bass as bass
import concourse.tile as tile
from concourse import bacc, mybir

f32 = mybir.dt.float32
i32 = mybir.dt.int32

P = 128
CORES = 8
L = 3
EPS = 1e-5


# ---------------------------------------------------------------- host prep

def _prep(x, edge_index):
    """Bucket edges by (core, chunk), append self-edges, pad to 128-tiles.

    Returns per-core arrays + the chunk tile counts (shared across cores).
    """
    N, D = x.shape
    E = edge_index.shape[1]
    n_own = (N + CORES - 1) // CORES            # nodes per core (last short)
    n_pad = ((n_own + P - 1) // P) * P          # padded to chunk multiple
    n_chunks = n_pad // P

    src = edge_index[0].astype(np.int64)
    dst = edge_index[1].astype(np.int64)

    # global padded-table row of node n (tables are [CORES*n_pad, D])
    def table_row(n):
        c = n // n_own
        return c * n_pad + (n - c * n_own)

    core_of = dst // n_own
    local = dst - core_of * n_own
    chunk_of = local // P
    dst_rel = local % P

    # self-edges: every real node, plus pad slots (src -> own row 0) so that
    # deg >= 1 everywhere and no inf/NaN enters the pipeline
    counts = np.zeros((CORES, n_chunks), np.int64)
    np.add.at(counts, (core_of, chunk_of), 1)
    counts += P  # one self-edge per slot in every chunk (incl. pad slots)

    tiles_per_chunk = ((counts.max(axis=0) + P - 1) // P).astype(np.int64)
    tile_base = np.concatenate([[0], np.cumsum(tiles_per_chunk)])
    NT = int(tile_base[-1])

    src_arr = np.zeros((CORES, P, NT), np.int32)      # table rows to gather
    rel_arr = np.full((CORES, P, NT), -1.0, np.float32)  # dst col or -1

    fill = np.zeros((CORES, n_chunks), np.int64)

    def put(c, ch, s_row, r):
        j = fill[c, ch]
        fill[c, ch] = j + 1
        t = tile_base[ch] + j // P
        p = j % P
        src_arr[c, p, t] = s_row
        rel_arr[c, p, t] = r

    # self-edges first (also covers pad slots)
    for c in range(CORES):
        base = c * n_own
        for ch in range(n_chunks):
            for r in range(P):
                n_local = ch * P + r
                if base + n_local < N and n_local < n_own:
                    put(c, ch, c * n_pad + n_local, r)
                else:
                    put(c, ch, c * n_pad, r)  # pad slot: gather own row 0
    # real edges (vectorized fill)
    order = np.lexsort((chunk_of, core_of))
    so, co, cho, dro = (src[order], core_of[order], chunk_of[order],
                        dst_rel[order])
    rows = table_row(so)
    grp = co * n_chunks + cho
    # positions within each (core, chunk) group, offset by current fill
    starts = np.searchsorted(grp, np.arange(CORES * n_chunks))
    pos = np.arange(E) - starts[grp] + fill.ravel()[grp]
    t_idx = tile_base[cho] + pos // P
    p_idx = pos % P
    src_arr[co, p_idx, t_idx] = rows
    rel_arr[co, p_idx, t_idx] = dro

    # per-core transposed, padded inputs
    xT = np.zeros((CORES, D, n_pad), np.float32)
    for c in range(CORES):
        lo, hi = c * n_own, min((c + 1) * n_own, N)
        xT[c, :, :hi - lo] = x[lo:hi].T
    return (xT, src_arr, rel_arr, tiles_per_chunk.astype(int), n_own, n_pad,
            n_chunks, NT)


# ------------------------------------------------------------- device build

def _build(D, n_pad, n_chunks, NT, tiles_per_chunk, n_real_last, N_total):
    """Build the SPMD Bass program (same for all cores)."""
    import os
    STAGE = int(os.environ.get("KERNEL_STAGE", "4"))
    nc = bacc.Bacc("TRN2", target_bir_lowering=False, debug=False,
                   num_devices=CORES)
    TBL = CORES * n_pad

    xT_in = nc.dram_tensor("xT_in", [D, n_pad], f32, kind="ExternalInput")
    src_in = nc.dram_tensor("src_in", [P, NT], i32, kind="ExternalInput")
    rel_in = nc.dram_tensor("rel_in", [P, NT], f32, kind="ExternalInput")
    Ws_in = nc.dram_tensor("Ws_in", [L * D, D], f32, kind="ExternalInput")
    gb_in = nc.dram_tensor("gb_in", [D, 2 * L], f32, kind="ExternalInput")
    out_ext = nc.dram_tensor("out", [D, n_pad], f32, kind="ExternalOutput")

    h_own = nc.dram_tensor("h_own", [n_pad, D], f32)
    h_tbl = nc.dram_tensor("h_tbl", [TBL, D], f32)
    h_gat = nc.dram_tensor("h_gat", [TBL, D], f32, kind="ExternalOutput")
    st_in = nc.dram_tensor("st_in", [P, 2], f32)
    st_out = nc.dram_tensor("st_out", [P, 2], f32)

    RG = [list(range(CORES))]
    AOP = mybir.AluOpType

    with tile.TileContext(nc) as tc:
        with tc.tile_pool(name="big", bufs=1) as big, \
             tc.tile_pool(name="sm", bufs=1) as sm, \
             tc.tile_pool(name="gat", bufs=14) as gat, \
             tc.tile_pool(name="oh", bufs=8) as ohp, \
             tc.tile_pool(name="work", bufs=3) as wk, \
             tc.tile_pool(name="ps", bufs=2, space="PSUM") as ps, \
             tc.tile_pool(name="psd", bufs=2, space="PSUM") as psd:

            # ---------------- persistent SBUF state
            xT = big.tile([D, n_pad], f32)
            nc.sync.dma_start(out=xT[:], in_=xT_in[:, :])
            src_sb = big.tile([P, NT], i32)
            nc.sync.dma_start(out=src_sb[:], in_=src_in[:, :])
            rel_sb = big.tile([P, NT], f32)
            nc.sync.dma_start(out=rel_sb[:], in_=rel_in[:, :])
            Ws_sb = sm.tile([D, L * D], f32)
            for i in range(L):
                nc.sync.dma_start(out=Ws_sb[:, i * D:(i + 1) * D],
                                  in_=Ws_in[i * D:(i + 1) * D, :])
            gb_sb = sm.tile([D, 2 * L], f32)
            nc.sync.dma_start(out=gb_sb[:], in_=gb_in[:, :])

            iota_i = sm.tile([P, P], i32)
            nc.gpsimd.iota(iota_i[:], pattern=[[1, P]], base=0,
                           channel_multiplier=0)
            iota_f = sm.tile([P, P], f32)
            nc.vector.tensor_copy(iota_f[:], iota_i[:])
            iota_col_i = sm.tile([P, P], i32)
            nc.gpsimd.iota(iota_col_i[:], pattern=[[1, P]], base=0,
                           channel_multiplier=1)
            iota_col = sm.tile([P, 1], f32)
            nc.vector.tensor_copy(iota_col[:], iota_col_i[:, 0:1])
            ones_col = sm.tile([P, 1], f32)
            nc.vector.memset(ones_col[:], 1.0)
            ones_sq = sm.tile([P, P], f32)
            nc.vector.memset(ones_sq[:], 1.0)

            dis_col = sm.tile([P, n_chunks], f32)   # dis, node-major cols
            dis_bc = big.tile([P, n_pad], f32)      # dis bcast over rows
            agg = big.tile([D, n_pad], f32)         # aggT per layer
            n_own_cols = (n_chunks - 1) * P + n_real_last
            if n_own_cols < n_pad:
                nc.vector.memset(agg[:, n_own_cols:], 0.0)
            slots = sm.tile([P, 2 * n_chunks], f32)  # per-chunk sums/sumsq
            stat = sm.tile([P, 8], f32)              # small scratch columns

            tb = np.concatenate([[0], np.cumsum(tiles_per_chunk)]).astype(int)

            # ---------------- one-time: degrees -> dis -> dis broadcast
            for ch in range(n_chunks):
                dps = psd.tile([P, 1], f32, space="PSUM", tag="deg")
                for t in range(tb[ch], tb[ch + 1]):
                    oht = ohp.tile([P, P], f32, tag="oh")
                    nc.vector.tensor_scalar(
                        out=oht[:], in0=iota_f[:],
                        scalar1=rel_sb[:, t:t + 1], scalar2=None,
                        op0=AOP.is_equal)
                    nc.tensor.matmul(out=dps[:], lhsT=oht[:], rhs=ones_col[:],
                                     start=(t == tb[ch]),
                                     stop=(t == tb[ch + 1] - 1))
                # dis = 1/sqrt(deg)
                nc.vector.reciprocal(stat[:, 0:1], dps[:])
                nc.scalar.sqrt(dis_col[:, ch:ch + 1], stat[:, 0:1])
                # dis broadcast to all partitions: ones128 @ diag(dis)
                diag = wk.tile([P, P], f32, tag="diag")
                nc.vector.tensor_scalar(
                    out=diag[:], in0=iota_f[:], scalar1=iota_col[:],
                    scalar2=dis_col[:, ch:ch + 1],
                    op0=AOP.is_equal, op1=AOP.mult)
                bps = psd.tile([P, P], f32, space="PSUM", tag="bc")
                nc.tensor.matmul(out=bps[:], lhsT=ones_sq[:], rhs=diag[:],
                                 start=True, stop=True)
                nc.scalar.copy(dis_bc[:, ch * P:(ch + 1) * P], bps[:])

            # ---------------- layers
            inv_n = 1.0 / float(N_total)
            if STAGE == 1:
                nc.scalar.copy(xT[:, 0:n_chunks], dis_col[:, 0:n_chunks])
            for i in range(range(0) and 0 or (L if STAGE >= 2 else 0)):
                # h' = dis * (x @ W_i), written row-major into own table rows
                for ch in range(n_chunks):
                    hps = ps.tile([P, D], f32, space="PSUM", tag="h")
                    nc.tensor.matmul(out=hps[:],
                                     lhsT=xT[:, ch * P:(ch + 1) * P],
                                     rhs=Ws_sb[:, i * D:(i + 1) * D],
                                     start=True, stop=True)
                    hsb = wk.tile([P, D], f32, tag="hsb")
                    nc.scalar.mul(out=hsb[:], in_=hps[:],
                                  mul=dis_col[:, ch:ch + 1])
                    nc.sync.dma_start(out=h_own[ch * P:(ch + 1) * P, :],
                                      in_=hsb[:])
                nc.gpsimd.collective_compute(
                    "AllGather", AOP.bypass, replica_groups=RG,
                    ins=[h_own[:, :]], outs=[h_tbl[:, :]])
                nc.sync.dma_start(out=h_gat[:, :], in_=h_tbl[:, :])
                if STAGE == 2:
                    continue

                # edge phase: gather + one-hot matmul, chunk accumulation
                for ch in range(n_chunks):
                    aps = ps.tile([D, P], f32, space="PSUM", tag="agg")
                    for t in range(tb[ch], tb[ch + 1]):
                        g = gat.tile([P, D], f32, tag="g")
                        nc.gpsimd.indirect_dma_start(
                            out=g[:], out_offset=None,
                            in_=h_gat[:, :],
                            in_offset=bass.IndirectOffsetOnAxis(
                                ap=src_sb[:, t:t + 1], axis=0))
                        oht = ohp.tile([P, P], f32, tag="oh")
                        nc.vector.tensor_scalar(
                            out=oht[:], in0=iota_f[:],
                            scalar1=rel_sb[:, t:t + 1], scalar2=None,
                            op0=AOP.is_equal)
                        nc.tensor.matmul(out=aps[:], lhsT=g[:], rhs=oht[:],
                                         start=(t == tb[ch]),
                                         stop=(t == tb[ch + 1] - 1))
                    # column scale by dis_dst; accumulate BN sums
                    w = P if ch < n_chunks - 1 else n_real_last
                    nc.vector.tensor_tensor(
                        out=agg[:, ch * P:ch * P + w],
                        in0=aps[:, 0:w],
                        in1=dis_bc[:, ch * P:ch * P + w],
                        op=AOP.mult)
                    nc.vector.tensor_reduce(
                        out=slots[:, ch:ch + 1],
                        in_=agg[:, ch * P:ch * P + w],
                        axis=mybir.AxisListType.X, op=AOP.add)
                    sq = wk.tile([P, P], f32, tag="sq")
                    nc.vector.tensor_tensor(
                        out=sq[:, 0:w], in0=agg[:, ch * P:ch * P + w],
                        in1=agg[:, ch * P:ch * P + w], op=AOP.mult)
                    nc.vector.tensor_reduce(
                        out=slots[:, n_chunks + ch:n_chunks + ch + 1],
                        in_=sq[:, 0:w],
                        axis=mybir.AxisListType.X, op=AOP.add)

                if STAGE == 3:
                    for ch in range(n_chunks):
                        s = slice(ch * P, (ch + 1) * P)
                        nc.vector.tensor_copy(xT[:, s], agg[:, s])
                    continue
                # stats: reduce chunk slots, AllReduce, scale/shift
                nc.vector.tensor_reduce(
                    out=stat[:, 0:1], in_=slots[:, 0:n_chunks],
                    axis=mybir.AxisListType.X, op=AOP.add)
                nc.vector.tensor_reduce(
                    out=stat[:, 1:2], in_=slots[:, n_chunks:2 * n_chunks],
                    axis=mybir.AxisListType.X, op=AOP.add)
                sin = wk.tile([P, 2], f32, tag="stin")
                nc.vector.tensor_copy(sin[:], stat[:, 0:2])
                nc.sync.dma_start(out=st_in[:, :], in_=sin[:])
                nc.gpsimd.collective_compute(
                    "AllReduce", AOP.add, replica_groups=RG,
                    ins=[st_in[:, :]], outs=[st_out[:, :]])
                sout = wk.tile([P, 2], f32, tag="stout")
                nc.sync.dma_start(out=sout[:], in_=st_out[:, :])
                # mean, var, scale = gamma*rsqrt(var+eps), shift = beta-sc*mean
                nc.vector.tensor_scalar(out=stat[:, 2:3], in0=sout[:, 0:1],
                                        scalar1=inv_n, scalar2=None,
                                        op0=AOP.mult)           # mean
                nc.vector.tensor_scalar(out=stat[:, 3:4], in0=sout[:, 1:2],
                                        scalar1=inv_n, scalar2=None,
                                        op0=AOP.mult)           # E[x^2]
                nc.vector.tensor_tensor(out=stat[:, 4:5], in0=stat[:, 2:3],
                                        in1=stat[:, 2:3], op=AOP.mult)
                nc.vector.tensor_tensor(out=stat[:, 4:5], in0=stat[:, 3:4],
                                        in1=stat[:, 4:5], op=AOP.subtract)
                nc.vector.tensor_scalar(out=stat[:, 4:5], in0=stat[:, 4:5],
                                        scalar1=float(EPS), scalar2=None,
                                        op0=AOP.add)            # var+eps
                nc.vector.reciprocal(stat[:, 5:6], stat[:, 4:5])
                nc.scalar.sqrt(stat[:, 6:7], stat[:, 5:6])      # rsqrt
                nc.vector.tensor_tensor(out=stat[:, 6:7],
                                        in0=gb_sb[:, 2 * i:2 * i + 1],
                                        in1=stat[:, 6:7], op=AOP.mult)
                nc.vector.tensor_tensor(out=stat[:, 7:8], in0=stat[:, 6:7],
                                        in1=stat[:, 2:3], op=AOP.mult)
                nc.vector.tensor_tensor(out=stat[:, 7:8],
                                        in0=gb_sb[:, 2 * i + 1:2 * i + 2],
                                        in1=stat[:, 7:8], op=AOP.subtract)

                # y = func(scale*agg + shift); x = y or x + y
                func = (mybir.ActivationFunctionType.Relu if i < L - 1
                        else mybir.ActivationFunctionType.Identity)
                for ch in range(n_chunks):
                    s = slice(ch * P, (ch + 1) * P)
                    if i == 0:
                        nc.scalar.activation(out=xT[:, s], in_=agg[:, s],
                                             func=func, bias=stat[:, 7:8],
                                             scale=stat[:, 6:7])
                    else:
                        yt = wk.tile([D, P], f32, tag="y")
                        nc.scalar.activation(out=yt[:], in_=agg[:, s],
                                             func=func, bias=stat[:, 7:8],
                                             scale=stat[:, 6:7])
                        nc.vector.tensor_tensor(out=xT[:, s], in0=xT[:, s],
                                                in1=yt[:], op=AOP.add)

            nc.sync.dma_start(out=out_ext[:, :], in_=xT[:])
    nc.compile()
    return nc


# ------------------------------------------------------------------ runner

class _Runner:
    """Persistent-jit PJRT runner (run_bass_via_pjrt, callable repeatedly)."""

    def __init__(self, nc, n_cores):
        import jax
        from jax.experimental.shard_map import shard_map
        from jax.sharding import Mesh, PartitionSpec
        from concourse import bass2jax
        self.jax = jax
        bass2jax.install_neuronx_cc_hook()
        in_names, out_names, out_avals, zero_outs = [], [], [], []
        partition_name = (nc.partition_id_tensor.name
                          if nc.partition_id_tensor else None)
        for alloc in nc.m.functions[0].allocations:
            if not isinstance(alloc, mybir.MemoryLocationSet):
                continue
            name = alloc.memorylocations[0].name
            if alloc.kind == "ExternalInput":
                if name != partition_name:
                    in_names.append(name)
            elif alloc.kind == "ExternalOutput":
                out_names.append(name)
                shape = tuple(alloc.tensor_shape)
                dtype = mybir.dt.np(alloc.dtype)
                out_avals.append(jax.core.ShapedArray(shape, dtype))
                zero_outs.append(np.zeros(shape, dtype))
        self.in_names, self.out_names = in_names, out_names
        self.out_avals, self.zero_outs = out_avals, zero_outs
        n_params, n_outs = len(in_names), len(out_avals)
        all_in = list(in_names) + list(out_names)
        if partition_name is not None:
            all_in.append(partition_name)
        from concourse.bass2jax import _bass_exec_p, partition_id_tensor

        def _body(*args):
            operands = list(args)
            if partition_name is not None:
                operands.append(partition_id_tensor())
            outs = _bass_exec_p.bind(
                *operands, out_avals=tuple(out_avals),
                in_names=tuple(all_in), out_names=tuple(out_names),
                lowering_input_output_aliases=(),
                sim_require_finite=False, sim_require_nnan=False, nc=nc)
            return tuple(outs)

        devices = jax.devices()[:n_cores]
        self.n_cores = n_cores
        self.mesh = Mesh(np.asarray(devices), ("core",))
        in_specs = (PartitionSpec("core"),) * (n_params + n_outs)
        out_specs = (PartitionSpec("core"),) * len(out_names)
        self.fn = jax.jit(
            shard_map(_body, mesh=self.mesh, in_specs=in_specs,
                      out_specs=out_specs, check_rep=False),
            keep_unused=True)
        self.dev_in = None

    def put(self, in_maps):
        from jax.sharding import NamedSharding, PartitionSpec
        sh = NamedSharding(self.mesh, PartitionSpec("core"))
        n = self.n_cores
        concat_in = [
            np.concatenate([np.asarray(in_maps[c][name]) for c in range(n)],
                           axis=0)
            for name in self.in_names]
        concat_zeros = [np.zeros((n * z.shape[0], *z.shape[1:]), z.dtype)
                        for z in self.zero_outs]
        self.dev_in = [self.jax.device_put(a, sh)
                       for a in concat_in + concat_zeros]
        self.jax.block_until_ready(self.dev_in)

    def __call__(self, fetch=("out",)):
        out = self.fn(*self.dev_in)
        self.jax.block_until_ready(out)
        n = self.n_cores
        return [
            {name: np.asarray(out[i]).reshape(n, *self.out_avals[i].shape)[c]
             for i, name in enumerate(self.out_names) if name in fetch}
            for c in range(n)]


_CACHE = {}


def _get_runner(N, D, tiles_per_chunk, n_own, n_pad, n_chunks, NT):
    key = (N, D, NT)
    if key in _CACHE:
        return _CACHE[key]
    n_real_last = n_own - (n_chunks - 1) * P
    nc = _build(D, n_pad, n_chunks, NT, tiles_per_chunk, n_real_last, N)
    r = _Runner(nc, CORES)
    _CACHE[key] = r
    return r


def kernel(x, edge_index, Ws, bs, gammas, betas):
    x = np.asarray(x, np.float32)
    edge_index = np.asarray(edge_index, np.int32)
    Ws = np.asarray(Ws, np.float32)
    gammas = np.asarray(gammas, np.float32)
    betas = np.asarray(betas, np.float32)
    N, D = x.shape

    (xT, src_arr, rel_arr, tpc, n_own, n_pad, n_chunks, NT) = _prep(
        x, edge_index)
    r = _get_runner(N, D, tpc, n_own, n_pad, n_chunks, NT)

    Ws_flat = Ws.reshape(L * D, D)
    gb = np.zeros((D, 2 * L), np.float32)
    for i in range(L):
        gb[:, 2 * i] = gammas[i]
        gb[:, 2 * i + 1] = betas[i]

    in_maps = [{"xT_in": xT[c], "src_in": src_arr[c], "rel_in": rel_arr[c],
                "Ws_in": Ws_flat, "gb_in": gb} for c in range(CORES)]
    r.put(in_maps)
    res = r()
    out = np.empty((N, D), np.float32)
    for c in range(CORES):
        lo, hi = c * n_own, min((c + 1) * n_own, N)
        out[lo:hi] = res[c]["out"][:, :hi - lo].T
    return out
